# revision 26
# baseline (speedup 1.0000x reference)
"""MLA (DeepSeek-style multi-head latent attention) Bass kernel for 8 trn2 NeuronCores.

Sharding: tensor-parallel over heads (2 heads/core) for the big projections +
attention. The low-rank A-projections are CHANNEL-sharded (each core computes
256 of the 2048 hq+kv latent channels for all 2048 tokens) so the AllGathered
latents read back as contiguous 4KB runs; k_pe (64 rope channels) is computed
redundantly on every core, skipping it in the collective. RMS normalization
happens after the gather: per-core partial sum-of-squares rides the AllGather
as one extra bf16 row per channel block, and the rsqrt factors are folded into
the projection PSUM->SBUF copies (free-dim broadcast for q/k, per-partition
tensor_scalar for v). The output projection is column-parallel with a split
AllGather: head 0's attention output gathers and partially accumulates into
wo while head 1's attention still runs.

All tensors bf16 (fp32 PSUM accumulation); exp softmax without max-subtract.
Host-side (free) preprocessing: bf16 casts, weight transposes/permutations,
folding q_norm_w/kv_norm_w and SOFTMAX_SCALE into wq_b/wkv_b, rope sign folds.
"""

import math
import sys

import numpy as np

for _p in ("/opt/trn_rl_repo", "/root/.axon_site/_ro/trn_rl_repo"):
    if _p not in sys.path:
        sys.path.append(_p)

B, S, H = 1, 2048, 2048
NH = 16
Q_LORA, KV_LORA = 1536, 512
D_NOPE, D_ROPE, D_V = 128, 64, 128
D_QK = D_NOPE + D_ROPE
ROPE_FACTOR, MSCALE = 4.0, 1.0
SOFTMAX_SCALE = D_QK ** -0.5 * (0.1 * MSCALE * math.log(ROPE_FACTOR) + 1.0) ** 2
EPS = 1e-6

NCORES = 8
SSH = S // NCORES          # 256 output channels per core (wo column-parallel)
CTOT = Q_LORA + KV_LORA + D_ROPE   # 2112 latent channels

_CACHE = {}


def _build(has_mask: bool):
    import concourse.bacc as bacc
    import concourse.mybir as mybir
    import concourse.tile as tile

    f32 = mybir.dt.float32
    b16 = mybir.dt.bfloat16
    AF = mybir.ActivationFunctionType
    OP = mybir.AluOpType

    nc = bacc.Bacc("TRN2", target_bir_lowering=False, debug=False,
                   num_devices=NCORES)

    hidT = nc.dram_tensor("hidT", [16, 128, S], b16, kind="ExternalInput")
    a_own = nc.dram_tensor("a_own", [16, 128, 256], b16, kind="ExternalInput")
    a_pe = nc.dram_tensor("a_pe", [16, 128, 64], b16, kind="ExternalInput")
    cosT2 = nc.dram_tensor("cosT2", [128, S], b16, kind="ExternalInput")
    sinT2s = nc.dram_tensor("sinT2s", [128, S], b16, kind="ExternalInput")
    wqbT = nc.dram_tensor("wqbT", [12, 128, 384], b16, kind="ExternalInput")
    wkvbT = nc.dram_tensor("wkvbT", [4, 128, 512], b16, kind="ExternalInput")
    woT = nc.dram_tensor("woT", [16, 128, SSH], b16, kind="ExternalInput")
    ones_a = nc.dram_tensor("ones_a", [128, 1], b16, kind="ExternalInput")
    ones_bf = nc.dram_tensor("ones_bf", [1, 128], f32, kind="ExternalInput")
    selH_d = nc.dram_tensor("selH", [16, 1], b16, kind="ExternalInput")
    selC_d = nc.dram_tensor("selC", [16, 1], b16, kind="ExternalInput")
    if has_mask:
        maskT = nc.dram_tensor("maskT", [S, S], b16, kind="ExternalInput")
    out = nc.dram_tensor("out", [S, SSH], f32, kind="ExternalOutput")

    bounce1 = [nc.dram_tensor(f"bounce1{i}", [129, S], b16)
               for i in range(2)]
    gath1 = [nc.dram_tensor(f"gath1{i}", [NCORES, 129, S], b16,
                            addr_space="Shared") for i in range(2)]
    bounce2a = nc.dram_tensor("bounce2a", [128, S], b16)
    gath2a = nc.dram_tensor("gath2a", [NCORES, 128, S], b16,
                            addr_space="Shared")
    B2SPLIT = 1536
    bounce2b = [nc.dram_tensor("bounce2b0", [128, B2SPLIT], b16),
                nc.dram_tensor("bounce2b1", [128, S - B2SPLIT], b16)]
    gath2b = [nc.dram_tensor("gath2b0", [NCORES, 128, B2SPLIT], b16,
                             addr_space="Shared"),
              nc.dram_tensor("gath2b1", [NCORES, 128, S - B2SPLIT], b16,
                             addr_space="Shared")]

    RG = [list(range(NCORES))]

    def mm(ps, lhsT, rhs, start, stop):
        nc.tensor.matmul(ps, lhsT, rhs, start=start, stop=stop)

    from contextlib import ExitStack
    with tile.TileContext(nc) as tc, ExitStack() as _st:
        constp = _st.enter_context(tc.tile_pool(name="const", bufs=1))
        ones_col = constp.tile([128, 1], b16)
        nc.sync.dma_start(ones_col[:], ones_a.ap())
        ones_row = constp.tile([1, 128], f32)
        nc.sync.dma_start(ones_row[:], ones_bf.ap())
        selh_sb = constp.tile([16, 1], b16)
        nc.sync.dma_start(selh_sb[:], selH_d.ap())
        selc_sb = constp.tile([16, 1], b16)
        nc.sync.dma_start(selc_sb[:], selC_d.ap())
        eps1 = constp.tile([1, 1], f32)
        nc.any.memset(eps1[:], EPS)
        eps_col = constp.tile([128, 1], f32)
        nc.any.memset(eps_col[:], EPS)

        # stage-1 weight tiles (DMAs issued after stage-0's loads)
        s1wp = _st.enter_context(tc.tile_pool(name="s1w", bufs=1))
        wqb_sb = s1wp.tile([128, 12, 384], b16)
        wkvb_sb = s1wp.tile([128, 4, 512], b16)
        cos2_sb = s1wp.tile([128, S], b16)
        sin2_sb = s1wp.tile([128, S], b16)
        wot_sb = s1wp.tile([128, 16, SSH], b16)   # DMA issued before attention

        # attention-lifetime pool (also covers the wo epilogue)
        attp = _st.enter_context(tc.tile_pool(name="att", bufs=1))
        kpe_sb = attp.tile([128, S], b16)
        nc.any.memset(kpe_sb[64:128, :], 0.0)

        # ---------------- stage 0: own 256 latent channels for all tokens
        with tc.tile_pool(name="s0", bufs=1) as s0p, \
             tc.tile_pool(name="s0ps", bufs=3, space="PSUM") as s0ps, \
             tc.tile_pool(name="s0ss", bufs=2, space="PSUM") as s0ssp, \
             tc.tile_pool(name="s0pe", bufs=2, space="PSUM") as s0pe, \
             tc.tile_pool(name="s0sq", bufs=3) as s0sqp:
            a_sb = s0p.tile([128, 16, 256], b16)
            hid_sb = s0p.tile([128, 16, S], b16)
            for g in range(8):
                gs = slice(g * 2, (g + 1) * 2)
                nc.sync.dma_start(
                    a_sb[:, gs, :],
                    a_own.ap()[gs].rearrange("o p c -> p o c"))
                nc.sync.dma_start(
                    hid_sb[:, gs, :],
                    hidT.ap()[gs].rearrange("o p s -> p o s"))
            ape_sb = s0p.tile([128, 16, 64], b16)
            nc.sync.dma_start(ape_sb[:], a_pe.ap().rearrange("o p c -> p o c"))
            # stage-1 weights load behind stage-0's operands
            nc.sync.dma_start(cos2_sb[:], cosT2.ap())
            nc.sync.dma_start(sin2_sb[:], sinT2s.ap())
            nc.sync.dma_start(wqb_sb[:],
                              wqbT.ap().rearrange("o p d -> p o d"))
            nc.sync.dma_start(wkvb_sb[:],
                              wkvbT.ap().rearrange("o p d -> p o d"))

            raw = s0p.tile([128, 2, S], b16)
            kpraw = s0p.tile([64, S], b16)
            ssb16 = [s0p.tile([1, S], b16, name=f"ssb{i}")
                     for i in range(2)]
            for ct in range(2):
                for tt in range(4):
                    sl = slice(tt * 512, (tt + 1) * 512)
                    ps = s0ps.tile([128, 512], f32, tag="s0ps")
                    for hb in range(16):
                        mm(ps, a_sb[:, hb, ct * 128:(ct + 1) * 128],
                           hid_sb[:, hb, sl], hb == 0, hb == 15)
                    nc.vector.tensor_copy(raw[:, ct, sl], ps[:])
                    sq = s0sqp.tile([128, 512], b16, tag="s0sq")
                    nc.scalar.activation(sq[:], ps[:], AF.Square)
                    ssp = s0ssp.tile([1, 512], f32, tag="ss")
                    mm(ssp, ones_col, sq, True, True)
                    nc.scalar.copy(ssb16[ct][:, sl], ssp[:])
                    nc.sync.dma_start(
                        bounce1[ct].ap()[0:128, sl], raw[:, ct, sl])
                # ship this channel slab; the second overlaps consumption
                nc.sync.dma_start(bounce1[ct].ap()[128:129, :],
                                  ssb16[ct][:])
                nc.gpsimd.collective_compute(
                    "AllGather", OP.bypass, replica_groups=RG,
                    ins=[bounce1[ct].ap().opt()],
                    outs=[gath1[ct].ap().opt()])

            # k_pe matmuls after the ship: they fill the AllGather wait
            for tt in range(4):
                sl = slice(tt * 512, (tt + 1) * 512)
                kp = s0pe.tile([64, 512], f32, tag="kpe")
                for hb in range(16):
                    mm(kp, ape_sb[:, hb, :], hid_sb[:, hb, sl],
                       hb == 0, hb == 15)
                nc.vector.tensor_copy(kpraw[:, sl], kp[:])

            # k_pe rope (redundant on every core; not in the collective)
            t1 = s0p.tile([64, S], b16)
            nc.vector.tensor_tensor(t1[:], kpraw[:], cos2_sb[0:64, :], OP.mult)
            rsw = s0p.tile([64, S], b16)
            nc.sync.dma_start(rsw[0:32], kpraw[32:64])
            nc.sync.dma_start(rsw[32:64], kpraw[0:32])
            nc.vector.tensor_tensor(rsw[:], rsw[:], sin2_sb[0:64, :], OP.mult)
            nc.vector.tensor_tensor(kpe_sb[0:64, :], t1[:], rsw[:], OP.add)

        # ---------------- stage 1: per-head projections + attention
        qn0 = attp.tile([128, S], b16)
        qt1 = attp.tile([128, S], b16)
        qn1 = attp.tile([128, S], b16)
        qdst = (qn0, qt1, qn1)
        kn0 = attp.tile([128, S], b16)
        kn1 = attp.tile([128, S], b16)
        kn = (kn0, kn1)
        vt = [attp.tile([128, 256], b16, name=f"vt{tb}") for tb in range(16)]
        qt1r = attp.tile([128, S], b16)
        qr1 = attp.tile([128, S], b16)
        nc.any.memset(qr1[64:128, :], 0.0)
        xh0 = attp.tile([128, S], b16)
        xh1 = attp.tile([128, S], b16)

        with tc.tile_pool(name="proj", bufs=1) as prj, \
             tc.tile_pool(name="bcp", bufs=1) as bcp:
            # gathered reads: all contiguous 4KB runs
            # channel tile id: slab i row-block r -> global tile 2r+i.
            # hq_all/kv_all cc index == global tile id (hq 0..11, kv 0..3).
            partials = prj.tile([16, S], b16)
            hq_all = prj.tile([128, 12, S], b16)
            kv_all = prj.tile([128, 4, S], b16)
            for i in range(2):
                nc.sync.dma_start(
                    partials[8 * i:8 * i + 8, :],
                    gath1[i].ap()[:, 128, :])
                for r in (6, 7):
                    nc.sync.dma_start(
                        kv_all[:, 2 * (r - 6) + i, :],
                        gath1[i].ap()[r, 0:128, :])
                for r in range(6):
                    nc.sync.dma_start(
                        hq_all[:, 2 * r + i, :],
                        gath1[i].ap()[r, 0:128, :])

            # rms factors: [1, S] free-layout (q/k) + [128, 16] col-layout (v)
            sq_hq = prj.tile([1, S], f32)
            sq_kv = prj.tile([1, S], f32)
            rc_hq_t = prj.tile([1, S], f32)
            rc_kv_t = prj.tile([1, S], f32)
            sqcol = prj.tile([128, 16], f32)
            rckv_col = prj.tile([128, 16], f32)
            bc_hq, bc_kv = [], []
            with tc.tile_pool(name="pfac", bufs=2, space="PSUM") as pfac, \
                 tc.tile_pool(name="pbc", bufs=2, space="PSUM") as pbc:
                for tt in range(4):
                    sl = slice(tt * 512, (tt + 1) * 512)
                    for selt, sqt, rct in ((selh_sb, sq_hq, rc_hq_t),
                                           (selc_sb, sq_kv, rc_kv_t)):
                        ps2 = pfac.tile([1, 512], f32, tag="ps2")
                        mm(ps2, selt, partials[:, sl], True, True)
                        nc.scalar.activation(sqt[:, sl], ps2[:], AF.Sqrt,
                                             bias=eps1[:])
                        nc.vector.reciprocal(rct[:, sl], sqt[:, sl])
                pscol = pfac.tile([128, 16], f32, tag="pscol")
                for tb in range(16):
                    mm(pscol[:, tb:tb + 1],
                       partials[:, tb * 128:(tb + 1) * 128],
                       selc_sb, True, True)
                nc.scalar.activation(sqcol[:], pscol[:], AF.Sqrt,
                                     bias=eps_col[:])
                nc.vector.reciprocal(rckv_col[:], sqcol[:])

                # broadcast rsqrt factors to 128 partitions, per 512-tok chunk
                for tt in range(4):
                    sl = slice(tt * 512, (tt + 1) * 512)
                    for ty, rct, dst in ((0, rc_hq_t, bc_hq),
                                         (1, rc_kv_t, bc_kv)):
                        psb = pbc.tile([128, 512], f32, tag="pbc")
                        mm(psb, ones_row, rct[:, sl], True, True)
                        bt = bcp.tile([128, 512], f32, name=f"bc{ty}_{tt}")
                        nc.vector.tensor_copy(bt[:], psb[:])
                        dst.append(bt)

            # projections with normalization fused into the PSUM->SBUF step
            with tc.tile_pool(name="p1ps", bufs=3, space="PSUM") as p1ps, \
                 tc.tile_pool(name="p1psv", bufs=2, space="PSUM") as p1psv:
                QCC = [0, 2, 4, 6, 8, 10, 1, 3, 5, 7, 9, 11]
                KCC = [0, 2, 1, 3]
                for tt in range(4):
                    sl = slice(tt * 512, (tt + 1) * 512)
                    for m in range(3):
                        ps = p1ps.tile([128, 512], f32, tag="p1ps")
                        for j, cc in enumerate(QCC):
                            mm(ps, wqb_sb[:, cc, m * 128:(m + 1) * 128],
                               hq_all[:, cc, sl], j == 0, j == 11)
                        nc.vector.tensor_tensor(qdst[m][:, sl], ps[:],
                                                bc_hq[tt][:], OP.mult)
                    for kh in range(2):
                        ps = p1ps.tile([128, 512], f32, tag="p1ps")
                        for j, cc in enumerate(KCC):
                            mm(ps, wkvb_sb[:, cc, kh * 128:(kh + 1) * 128],
                               kv_all[:, cc, sl], j == 0, j == 3)
                        nc.vector.tensor_tensor(kn[kh][:, sl], ps[:],
                                                bc_kv[tt][:], OP.mult)
                for tb in range(16):
                    ps = p1psv.tile([128, 256], f32, tag="p1psv")
                    for j, cc in enumerate(KCC):
                        mm(ps, kv_all[:, cc, tb * 128:(tb + 1) * 128],
                           wkvb_sb[:, cc, 256:512], j == 0, j == 3)
                        # lhsT = latent chunk [c,t], rhs = v cols of wkv_b'^T
                    nc.vector.tensor_scalar_mul(vt[tb][:], ps[:],
                                                rckv_col[:, tb:tb + 1])

                # rope on q (both heads share qt1: rows 0:64 h0, 64:128 h1)
                tmp = prj.tile([128, S], b16)
                for b in (0, 64):
                    nc.sync.dma_start(tmp[b:b + 32], qt1[b + 32:b + 64])
                    nc.sync.dma_start(tmp[b + 32:b + 64], qt1[b:b + 32])
                nc.vector.tensor_tensor(qt1r[:], qt1[:], cos2_sb[:], OP.mult)
                nc.vector.tensor_tensor(tmp[:], tmp[:], sin2_sb[:], OP.mult)
                nc.vector.tensor_tensor(qt1r[:], qt1r[:], tmp[:], OP.add)
                # h1 rope rows to base-0 tile (rows 64: zero; kpe rows 64: 0)
                nc.sync.dma_start(qr1[0:64, :], qt1r[64:128])

        # wo weights prefetch (overlaps attention)
        nc.sync.dma_start(wot_sb[:], woT.ap().rearrange("o p s -> p o s"))

        # attention + split AllGather + column-parallel wo
        with tc.tile_pool(name="apss", bufs=3, space="PSUM") as apss, \
             tc.tile_pool(name="apsx", bufs=2, space="PSUM") as apsx, \
             tc.tile_pool(name="apsd", bufs=2, space="PSUM") as apsd, \
             tc.tile_pool(name="wops", bufs=1, space="PSUM") as wops, \
             tc.tile_pool(name="aex", bufs=4) as aexp, \
             tc.tile_pool(name="asm", bufs=2) as asmp, \
             tc.tile_pool(name="amk", bufs=2) as amkp, \
             tc.tile_pool(name="wop", bufs=1) as wop, \
             tc.tile_pool(name="woot", bufs=3) as wootp:

            LOOKAHEAD = 3

            def attend(h, xh, ship=None):
                qn_h = qn0 if h == 0 else qn1
                qr_h = qt1r if h == 0 else qr1

                def finish(sb, psx, psdt):
                    # softmax normalize for a finished chunk; emitted a few
                    # matmuls into the NEXT chunk so the in-order PE never
                    # stalls on the (slow, single-lane) reciprocal
                    sl = slice(sb * 512, (sb + 1) * 512)
                    rd = asmp.tile([1, 512], f32, tag="rd")
                    nc.vector.reciprocal(rd[:], psdt[0:1, :])
                    psb2 = apsd.tile([128, 512], f32, tag="dn")
                    mm(psb2, ones_row, rd, True, True)
                    rdb = asmp.tile([128, 512], f32, tag="rdb")
                    nc.vector.tensor_copy(rdb[:], psb2[:])
                    nc.vector.tensor_tensor(xh[:, sl], psx[:], rdb[:],
                                            OP.mult)
                    if ship is not None:
                        ship(sb)

                pending = None
                for sb in range(4):
                    sl = slice(sb * 512, (sb + 1) * 512)
                    psx = apsx.tile([128, 512], f32, tag="apsx")
                    psdt = apsd.tile([128, 512], f32, tag="dn")
                    psd = psdt[0:1, :]
                    exq = []

                    def consume(tb, ex):
                        mm(psx, vt[tb][:, h * 128:(h + 1) * 128], ex,
                           tb == 0, tb == 15)
                        mm(psd, ones_col, ex, tb == 0, tb == 15)

                    for tb in range(16):
                        tsl = slice(tb * 128, (tb + 1) * 128)
                        pss = apss.tile([128, 512], f32, tag="apss")
                        mm(pss, kn[h][:, tsl], qn_h[:, sl], True, False)
                        mm(pss, kpe_sb[:, tsl], qr_h[:, sl], False, True)
                        if has_mask:
                            mk = amkp.tile([128, 512], b16, tag="amk")
                            nc.sync.dma_start(
                                mk[:], maskT.ap()[tsl, sl])
                            nc.vector.tensor_tensor(pss[:], pss[:], mk[:],
                                                    OP.add)
                        ex = aexp.tile([128, 512], b16, tag="aex")
                        nc.scalar.activation(ex[:], pss[:], AF.Exp)
                        exq.append((tb, ex))
                        if tb == 4 and pending is not None:
                            finish(*pending)
                            pending = None
                        if len(exq) > LOOKAHEAD:
                            consume(*exq.pop(0))
                    for item in exq:
                        consume(*item)
                    pending = (sb, psx, psdt)
                finish(*pending)

            attend(0, xh0)
            nc.sync.dma_start(bounce2a.ap(), xh0[:])
            nc.gpsimd.collective_compute(
                "AllGather", OP.bypass, replica_groups=RG,
                ins=[bounce2a.ap().opt()], outs=[gath2a.ap().opt()])

            # head-1 attention runs while gath2a lands + wo half-accumulates
            def ship_h1(sb):
                if sb in (2, 3):
                    i = sb - 2
                    hs = slice(0, B2SPLIT) if i == 0 else slice(B2SPLIT, S)
                    nc.sync.dma_start(bounce2b[i].ap(), xh1[:, hs])
                    nc.gpsimd.collective_compute(
                        "AllGather", OP.bypass, replica_groups=RG,
                        ins=[bounce2b[i].ap().opt()],
                        outs=[gath2b[i].ap().opt()])

            attend(1, xh1, ship=ship_h1)

            xe_a = wop.tile([128, 8, S], b16)
            for r in range(NCORES):
                nc.sync.dma_start(xe_a[:, r, :], gath2a.ap()[r])
            opart = [wop.tile([128, SSH], f32, name=f"op{st}")
                     for st in range(16)]
            for st in range(16):
                pso = wops.tile([128, SSH], f32, tag="wops")
                for r in range(NCORES):
                    mm(pso, xe_a[:, r, st * 128:(st + 1) * 128],
                       wot_sb[:, 2 * r, :], r == 0, r == 7)
                nc.vector.tensor_copy(opart[st][:], pso[:])

            xe_b = wop.tile([128, 8, S], b16)
            for i in range(2):
                hs = slice(0, B2SPLIT) if i == 0 else slice(B2SPLIT, S)
                for r in range(NCORES):
                    nc.sync.dma_start(xe_b[:, r, hs], gath2b[i].ap()[r])
                for st in (range(12) if i == 0 else range(12, 16)):
                    pso = wops.tile([128, SSH], f32, tag="wops")
                    for r in range(NCORES):
                        mm(pso, xe_b[:, r, st * 128:(st + 1) * 128],
                           wot_sb[:, 2 * r + 1, :], r == 0, r == 7)
                    ot = wootp.tile([128, SSH], f32, tag="ot")
                    nc.vector.tensor_tensor(ot[:], pso[:], opart[st][:],
                                            OP.add)
                    nc.sync.dma_start(out.ap()[st * 128:(st + 1) * 128, :],
                                      ot[:])

    nc.compile()
    return nc


def _prep_inputs(hidden_states, cos, sin, attn_mask, wq_a, q_norm_w, wq_b,
                 wkv_a, kv_norm_w, wkv_b, wo, has_mask):
    import ml_dtypes
    bf = ml_dtypes.bfloat16

    def c(x):
        return np.ascontiguousarray(x.astype(bf))

    hid = np.asarray(hidden_states, np.float32)[0]          # [S, H]
    hidT = c(hid.T.reshape(16, 128, S))                     # [H, S]
    A_T = np.vstack([np.asarray(wq_a, np.float32),
                     np.asarray(wkv_a, np.float32)]).T      # [H, CTOT]
    a_pe = c(A_T[:, 2048:2112].reshape(16, 128, 64))

    cosT = np.asarray(cos, np.float32).T                    # [64, S]
    sinT = np.asarray(sin, np.float32).T
    sinTs = sinT.copy()
    sinTs[0:32] *= -1.0
    cosT2 = c(np.concatenate([cosT, cosT], 0))              # [128, S]
    sinT2s = c(np.concatenate([sinTs, sinTs], 0))

    wqb = np.asarray(wq_b, np.float32) * np.asarray(q_norm_w, np.float32)[None]
    wqb = wqb * SOFTMAX_SCALE
    wkvb = (np.asarray(wkv_b, np.float32)
            * np.asarray(kv_norm_w, np.float32)[None])
    woT_full = np.asarray(wo, np.float32).T                 # [NH*DV, H]

    qperm = np.r_[0:128, 128:192, 320:384, 192:320]
    kvperm = np.r_[0:128, 256:384, 128:256, 384:512]

    # partial-sumsq row p: slab i=p//8, core r=p%8 -> global channel tile
    # 2r+i (tiles 0..11 are hq, 12..15 kv); fold the 1/D mean into the
    # selector values
    selH = np.zeros((16, 1), np.float32)
    selC = np.zeros((16, 1), np.float32)
    for p in range(16):
        tile_id = 2 * (p % 8) + p // 8
        if tile_id < 12:
            selH[p, 0] = 1.0 / Q_LORA
        else:
            selC[p, 0] = 1.0 / KV_LORA

    in_maps = []
    for r in range(NCORES):
        m = {
            "hidT": hidT,
            "a_own": c(A_T[:, r * 256:(r + 1) * 256].reshape(16, 128, 256)),
            "a_pe": a_pe,
            "cosT2": cosT2,
            "sinT2s": sinT2s,
            "wqbT": c(wqb[r * 384:(r + 1) * 384].T[:, qperm]
                      .reshape(12, 128, 384)),
            "wkvbT": c(wkvb[r * 512:(r + 1) * 512].T[:, kvperm]
                       .reshape(4, 128, 512)),
            "woT": c(woT_full[:, r * SSH:(r + 1) * SSH].reshape(16, 128, SSH)),
            "ones_a": np.ones((128, 1), bf),
            "ones_bf": np.ones((1, 128), np.float32),
            "selH": c(selH),
            "selC": c(selC),
        }
        if has_mask:
            m["maskT"] = c(np.asarray(attn_mask, np.float32).T)
        in_maps.append(m)
    return in_maps


def kernel(**inputs):
    from concourse.bass_utils import run_bass_kernel_spmd

    has_mask = bool(np.any(np.asarray(inputs["attn_mask"])))
    if has_mask not in _CACHE:
        _CACHE[has_mask] = _build(has_mask)
    nc = _CACHE[has_mask]

    in_maps = _prep_inputs(has_mask=has_mask, **inputs)
    res = run_bass_kernel_spmd(nc, in_maps, list(range(NCORES))).results
    full = np.concatenate([res[r]["out"] for r in range(NCORES)], axis=1)
    return full.reshape(B, S, H).astype(np.float32)


# revision 29
# speedup vs baseline: 1.0221x; 1.0221x over previous
"""MLA (DeepSeek-style multi-head latent attention) Bass kernel for 8 trn2 NeuronCores.

Sharding: tensor-parallel over heads (2 heads/core) for the big projections +
attention. The low-rank A-projections are CHANNEL-sharded (each core computes
256 of the 2048 hq+kv latent channels for all 2048 tokens) so the AllGathered
latents read back as contiguous 4KB runs; k_pe (64 rope channels) is computed
redundantly on every core, skipping it in the collective. RMS normalization
happens after the gather: per-core partial sum-of-squares rides the AllGather
as one extra bf16 row per channel block, and the rsqrt factors are folded into
the projection PSUM->SBUF copies (free-dim broadcast for q/k, per-partition
tensor_scalar for v). The output projection is column-parallel with a split
AllGather: head 0's attention output gathers and partially accumulates into
wo while head 1's attention still runs.

All tensors bf16 (fp32 PSUM accumulation); exp softmax without max-subtract.
Host-side (free) preprocessing: bf16 casts, weight transposes/permutations,
folding q_norm_w/kv_norm_w and SOFTMAX_SCALE into wq_b/wkv_b, rope sign folds.
"""

import math
import sys

import numpy as np

for _p in ("/opt/trn_rl_repo", "/root/.axon_site/_ro/trn_rl_repo"):
    if _p not in sys.path:
        sys.path.append(_p)

B, S, H = 1, 2048, 2048
NH = 16
Q_LORA, KV_LORA = 1536, 512
D_NOPE, D_ROPE, D_V = 128, 64, 128
D_QK = D_NOPE + D_ROPE
ROPE_FACTOR, MSCALE = 4.0, 1.0
SOFTMAX_SCALE = D_QK ** -0.5 * (0.1 * MSCALE * math.log(ROPE_FACTOR) + 1.0) ** 2
EPS = 1e-6

NCORES = 8
SSH = S // NCORES          # 256 output channels per core (wo column-parallel)
CTOT = Q_LORA + KV_LORA + D_ROPE   # 2112 latent channels

_CACHE = {}


def _build(has_mask: bool):
    import concourse.bacc as bacc
    import concourse.mybir as mybir
    import concourse.tile as tile

    f32 = mybir.dt.float32
    b16 = mybir.dt.bfloat16
    AF = mybir.ActivationFunctionType
    OP = mybir.AluOpType

    nc = bacc.Bacc("TRN2", target_bir_lowering=False, debug=False,
                   num_devices=NCORES)

    hidT = nc.dram_tensor("hidT", [16, 128, S], b16, kind="ExternalInput")
    a_own = nc.dram_tensor("a_own", [16, 128, 256], b16, kind="ExternalInput")
    a_pe = nc.dram_tensor("a_pe", [16, 128, 64], b16, kind="ExternalInput")
    cosT2 = nc.dram_tensor("cosT2", [128, S], b16, kind="ExternalInput")
    sinT2s = nc.dram_tensor("sinT2s", [128, S], b16, kind="ExternalInput")
    wqbT = nc.dram_tensor("wqbT", [12, 128, 384], b16, kind="ExternalInput")
    wkvbT = nc.dram_tensor("wkvbT", [4, 128, 512], b16, kind="ExternalInput")
    woT = nc.dram_tensor("woT", [16, 128, SSH], b16, kind="ExternalInput")
    ones_a = nc.dram_tensor("ones_a", [128, 1], b16, kind="ExternalInput")
    ones_bf = nc.dram_tensor("ones_bf", [1, 128], f32, kind="ExternalInput")
    selH_d = nc.dram_tensor("selH", [16, 1], b16, kind="ExternalInput")
    eye_d = nc.dram_tensor("eye", [128, 128], b16, kind="ExternalInput")
    selC_d = nc.dram_tensor("selC", [16, 1], b16, kind="ExternalInput")
    if has_mask:
        maskT = nc.dram_tensor("maskT", [S, S], b16, kind="ExternalInput")
    out = nc.dram_tensor("out", [S, SSH], f32, kind="ExternalOutput")

    bounce1 = [nc.dram_tensor(f"bounce1{i}", [129, S], b16)
               for i in range(2)]
    gath1 = [nc.dram_tensor(f"gath1{i}", [NCORES, 129, S], b16,
                            addr_space="Shared") for i in range(2)]
    bounce2a = [nc.dram_tensor("bounce2a0", [128, 1536], b16),
                nc.dram_tensor("bounce2a1", [128, S - 1536], b16)]
    gath2a = [nc.dram_tensor("gath2a0", [NCORES, 128, 1536], b16,
                             addr_space="Shared"),
              nc.dram_tensor("gath2a1", [NCORES, 128, S - 1536], b16,
                             addr_space="Shared")]
    B2SPLIT = 1536
    bounce2b = [nc.dram_tensor("bounce2b0", [128, B2SPLIT], b16),
                nc.dram_tensor("bounce2b1", [128, S - B2SPLIT], b16)]
    gath2b = [nc.dram_tensor("gath2b0", [NCORES, 128, B2SPLIT], b16,
                             addr_space="Shared"),
              nc.dram_tensor("gath2b1", [NCORES, 128, S - B2SPLIT], b16,
                             addr_space="Shared")]

    RG = [list(range(NCORES))]

    def mm(ps, lhsT, rhs, start, stop):
        nc.tensor.matmul(ps, lhsT, rhs, start=start, stop=stop)

    from contextlib import ExitStack
    with tile.TileContext(nc) as tc, ExitStack() as _st:
        constp = _st.enter_context(tc.tile_pool(name="const", bufs=1))
        ones_col = constp.tile([128, 1], b16)
        nc.sync.dma_start(ones_col[:], ones_a.ap())
        ones_row = constp.tile([1, 128], f32)
        nc.sync.dma_start(ones_row[:], ones_bf.ap())
        selh_sb = constp.tile([16, 1], b16)
        nc.sync.dma_start(selh_sb[:], selH_d.ap())
        selc_sb = constp.tile([16, 1], b16)
        nc.sync.dma_start(selc_sb[:], selC_d.ap())
        eye_sb = constp.tile([128, 128], b16)
        nc.sync.dma_start(eye_sb[:], eye_d.ap())
        eps1 = constp.tile([1, 1], f32)
        nc.any.memset(eps1[:], EPS)
        eps_col = constp.tile([128, 1], f32)
        nc.any.memset(eps_col[:], EPS)

        # stage-1 weight tiles (DMAs issued after stage-0's loads)
        s1wp = _st.enter_context(tc.tile_pool(name="s1w", bufs=1))
        wqb_sb = s1wp.tile([128, 12, 384], b16)
        wkvb_sb = s1wp.tile([128, 4, 512], b16)
        cos2_sb = s1wp.tile([128, S], b16)
        sin2_sb = s1wp.tile([128, S], b16)
        wot_sb = s1wp.tile([128, 16, SSH], b16)   # DMA issued before attention

        # attention-lifetime pool (also covers the wo epilogue)
        attp = _st.enter_context(tc.tile_pool(name="att", bufs=1))
        kpe_sb = attp.tile([128, S], b16)
        nc.any.memset(kpe_sb[64:128, :], 0.0)

        # ---------------- stage 0: own 256 latent channels for all tokens
        with tc.tile_pool(name="s0", bufs=1) as s0p, \
             tc.tile_pool(name="s0ps", bufs=3, space="PSUM") as s0ps, \
             tc.tile_pool(name="s0ss", bufs=2, space="PSUM") as s0ssp, \
             tc.tile_pool(name="s0pe", bufs=2, space="PSUM") as s0pe, \
             tc.tile_pool(name="s0sq", bufs=3) as s0sqp:
            a_sb = s0p.tile([128, 16, 256], b16)
            hid_sb = s0p.tile([128, 16, S], b16)
            for g in range(8):
                gs = slice(g * 2, (g + 1) * 2)
                nc.sync.dma_start(
                    a_sb[:, gs, :],
                    a_own.ap()[gs].rearrange("o p c -> p o c"))
                nc.sync.dma_start(
                    hid_sb[:, gs, :],
                    hidT.ap()[gs].rearrange("o p s -> p o s"))
            ape_sb = s0p.tile([128, 16, 64], b16)
            nc.sync.dma_start(ape_sb[:], a_pe.ap().rearrange("o p c -> p o c"))
            # stage-1 weights load behind stage-0's operands
            nc.sync.dma_start(cos2_sb[:], cosT2.ap())
            nc.sync.dma_start(sin2_sb[:], sinT2s.ap())
            nc.sync.dma_start(wqb_sb[:],
                              wqbT.ap().rearrange("o p d -> p o d"))
            nc.sync.dma_start(wkvb_sb[:],
                              wkvbT.ap().rearrange("o p d -> p o d"))

            raw = s0p.tile([128, 2, S], b16)
            kpraw = s0p.tile([64, S], b16)
            ssb16 = [s0p.tile([1, S], b16, name=f"ssb{i}")
                     for i in range(2)]
            for ct in range(2):
                for tt in range(4):
                    sl = slice(tt * 512, (tt + 1) * 512)
                    ps = s0ps.tile([128, 512], f32, tag="s0ps")
                    for hb in range(16):
                        mm(ps, a_sb[:, hb, ct * 128:(ct + 1) * 128],
                           hid_sb[:, hb, sl], hb == 0, hb == 15)
                    nc.vector.tensor_copy(raw[:, ct, sl], ps[:])
                    sq = s0sqp.tile([128, 512], b16, tag="s0sq")
                    nc.scalar.activation(sq[:], ps[:], AF.Square)
                    ssp = s0ssp.tile([1, 512], f32, tag="ss")
                    mm(ssp, ones_col, sq, True, True)
                    nc.scalar.copy(ssb16[ct][:, sl], ssp[:])
                    nc.sync.dma_start(
                        bounce1[ct].ap()[0:128, sl], raw[:, ct, sl])
                # ship this channel slab; the second overlaps consumption
                nc.sync.dma_start(bounce1[ct].ap()[128:129, :],
                                  ssb16[ct][:])
                nc.gpsimd.collective_compute(
                    "AllGather", OP.bypass, replica_groups=RG,
                    ins=[bounce1[ct].ap().opt()],
                    outs=[gath1[ct].ap().opt()])

            # k_pe matmuls after the ship: they fill the AllGather wait
            for tt in range(4):
                sl = slice(tt * 512, (tt + 1) * 512)
                kp = s0pe.tile([64, 512], f32, tag="kpe")
                for hb in range(16):
                    mm(kp, ape_sb[:, hb, :], hid_sb[:, hb, sl],
                       hb == 0, hb == 15)
                nc.vector.tensor_copy(kpraw[:, sl], kp[:])

            # k_pe rope (redundant on every core; not in the collective)
            t1 = s0p.tile([64, S], b16)
            nc.vector.tensor_tensor(t1[:], kpraw[:], cos2_sb[0:64, :], OP.mult)
            rsw = s0p.tile([64, S], b16)
            nc.sync.dma_start(rsw[0:32], kpraw[32:64])
            nc.sync.dma_start(rsw[32:64], kpraw[0:32])
            nc.vector.tensor_tensor(rsw[:], rsw[:], sin2_sb[0:64, :], OP.mult)
            nc.vector.tensor_tensor(kpe_sb[0:64, :], t1[:], rsw[:], OP.add)

        # ---------------- stage 1: per-head projections + attention
        qn0 = attp.tile([128, S], b16)
        qt1 = attp.tile([128, S], b16)
        qn1 = attp.tile([128, S], b16)
        qdst = (qn0, qt1, qn1)
        kn0 = attp.tile([128, S], b16)
        kn1 = attp.tile([128, S], b16)
        kn = (kn0, kn1)
        vt = [attp.tile([128, 256], b16, name=f"vt{tb}") for tb in range(16)]
        qt1r = attp.tile([128, S], b16)
        qr1 = attp.tile([128, S], b16)
        nc.any.memset(qr1[64:128, :], 0.0)
        xh0 = attp.tile([128, S], b16)
        xh1 = attp.tile([128, S], b16)

        with tc.tile_pool(name="proj", bufs=1) as prj, \
             tc.tile_pool(name="bcp", bufs=2) as bcp:
            # gathered reads: all contiguous 4KB runs
            # channel tile id: slab i row-block r -> global tile 2r+i.
            # hq_all/kv_all cc index == global tile id (hq 0..11, kv 0..3).
            partials = prj.tile([16, S], b16)
            hq_all = prj.tile([128, 12, S], b16)
            kv_all = prj.tile([128, 4, S], b16)
            for i in range(2):
                nc.sync.dma_start(
                    partials[8 * i:8 * i + 8, :],
                    gath1[i].ap()[:, 128, :])
                for r in (6, 7):
                    nc.sync.dma_start(
                        kv_all[:, 2 * (r - 6) + i, :],
                        gath1[i].ap()[r, 0:128, :])
                for r in range(6):
                    nc.sync.dma_start(
                        hq_all[:, 2 * r + i, :],
                        gath1[i].ap()[r, 0:128, :])

            # projections with normalization fused into the PSUM->SBUF step
            with tc.tile_pool(name="p1ps", bufs=3, space="PSUM") as p1ps, \
                 tc.tile_pool(name="p1psv", bufs=1, space="PSUM") as p1psv:
                QEVEN = [0, 2, 4, 6, 8, 10]
                QODD = [1, 3, 5, 7, 9, 11]
                KCC = [0, 2, 1, 3]
                # pass A: even channel tiles (slab 0) accumulate to bf16
                # partials while slab 1 is still gathering
                qparts = {}
                for tt in range(4):
                    sl = slice(tt * 512, (tt + 1) * 512)
                    for m in range(3):
                        ps = p1ps.tile([128, 512], f32, tag="p1ps")
                        for j, cc in enumerate(QEVEN):
                            mm(ps, wqb_sb[:, cc, m * 128:(m + 1) * 128],
                               hq_all[:, cc, sl], j == 0, j == 5)
                        qp = prj.tile([128, 512], b16, name=f"qp{tt}_{m}")
                        nc.vector.tensor_copy(qp[:], ps[:])
                        qparts[(tt, m)] = qp

            # rms factors: [1, S] free-layout (q/k) + [128, 16] col-layout (v)
                rc_hq_t = prj.tile([1, S], f32)
                rc_kv_t = prj.tile([1, S], f32)
                sqcol = prj.tile([128, 16], f32)
                rckv_col = prj.tile([128, 16], f32)
                with tc.tile_pool(name="sqp", bufs=2) as sqp, \
                     tc.tile_pool(name="pfac", bufs=2, space="PSUM") as pfac:
                    for tt in range(4):
                        sl = slice(tt * 512, (tt + 1) * 512)
                        for selt, rct in ((selh_sb, rc_hq_t),
                                          (selc_sb, rc_kv_t)):
                            ps2 = pfac.tile([1, 512], f32, tag="ps2")
                            mm(ps2, selt, partials[:, sl], True, True)
                            sqt = sqp.tile([1, 512], f32, tag="sq")
                            nc.scalar.activation(sqt[:], ps2[:], AF.Sqrt,
                                                 bias=eps1[:])
                            nc.vector.reciprocal(rct[:, sl], sqt[:])
                    pscol = pfac.tile([128, 16], f32, tag="pscol")
                    for tb in range(16):
                        mm(pscol[:, tb:tb + 1],
                           partials[:, tb * 128:(tb + 1) * 128],
                           selc_sb, True, True)
                    nc.scalar.activation(sqcol[:], pscol[:], AF.Sqrt,
                                         bias=eps_col[:])
                    nc.vector.reciprocal(rckv_col[:], sqcol[:])

                # pass B: identity re-injection + odd channel tiles, with
                # the rsqrt broadcasts produced on demand per token chunk
                with tc.tile_pool(name="pbc", bufs=2, space="PSUM") as pbc:
                    for tt in range(4):
                        sl = slice(tt * 512, (tt + 1) * 512)
                        bts = []
                        for ty, rct in ((0, rc_hq_t), (1, rc_kv_t)):
                            psb = pbc.tile([128, 512], f32, tag="pbc")
                            mm(psb, ones_row, rct[:, sl], True, True)
                            bt = bcp.tile([128, 512], f32, tag=f"bc{ty}")
                            nc.vector.tensor_copy(bt[:], psb[:])
                            bts.append(bt)
                        bchq_t, bckv_t = bts
                        for m in range(3):
                            ps = p1ps.tile([128, 512], f32, tag="p1ps")
                            mm(ps, eye_sb, qparts[(tt, m)][:], True, False)
                            for j, cc in enumerate(QODD):
                                mm(ps, wqb_sb[:, cc, m * 128:(m + 1) * 128],
                                   hq_all[:, cc, sl], False, j == 5)
                            nc.vector.tensor_tensor(qdst[m][:, sl], ps[:],
                                                    bchq_t[:], OP.mult)
                        for kh in range(2):
                            ps = p1ps.tile([128, 512], f32, tag="p1ps")
                            for j, cc in enumerate(KCC):
                                mm(ps, wkvb_sb[:, cc,
                                               kh * 128:(kh + 1) * 128],
                                   kv_all[:, cc, sl], j == 0, j == 3)
                            nc.vector.tensor_tensor(kn[kh][:, sl], ps[:],
                                                    bckv_t[:], OP.mult)
                for tb in range(16):
                    ps = p1psv.tile([128, 256], f32, tag="p1psv")
                    for j, cc in enumerate(KCC):
                        mm(ps, kv_all[:, cc, tb * 128:(tb + 1) * 128],
                           wkvb_sb[:, cc, 256:512], j == 0, j == 3)
                        # lhsT = latent chunk [c,t], rhs = v cols of wkv_b'^T
                    nc.vector.tensor_scalar_mul(vt[tb][:], ps[:],
                                                rckv_col[:, tb:tb + 1])

                # rope on q (both heads share qt1: rows 0:64 h0, 64:128 h1)
                tmp = xh1  # scratch; attention writes xh1 later
                for b in (0, 64):
                    nc.sync.dma_start(tmp[b:b + 32], qt1[b + 32:b + 64])
                    nc.sync.dma_start(tmp[b + 32:b + 64], qt1[b:b + 32])
                nc.vector.tensor_tensor(qt1r[:], qt1[:], cos2_sb[:], OP.mult)
                nc.vector.tensor_tensor(tmp[:], tmp[:], sin2_sb[:], OP.mult)
                nc.vector.tensor_tensor(qt1r[:], qt1r[:], tmp[:], OP.add)
                # h1 rope rows to base-0 tile (rows 64: zero; kpe rows 64: 0)
                nc.sync.dma_start(qr1[0:64, :], qt1r[64:128])

        # wo weights prefetch (overlaps attention)
        nc.sync.dma_start(wot_sb[:], woT.ap().rearrange("o p s -> p o s"))

        # attention + split AllGather + column-parallel wo
        with tc.tile_pool(name="apss", bufs=3, space="PSUM") as apss, \
             tc.tile_pool(name="apsx", bufs=2, space="PSUM") as apsx, \
             tc.tile_pool(name="apsd", bufs=2, space="PSUM") as apsd, \
             tc.tile_pool(name="wops", bufs=1, space="PSUM") as wops, \
             tc.tile_pool(name="aex", bufs=4) as aexp, \
             tc.tile_pool(name="asm", bufs=2) as asmp, \
             tc.tile_pool(name="amk", bufs=2) as amkp, \
             tc.tile_pool(name="wop", bufs=1) as wop, \
             tc.tile_pool(name="woot", bufs=3) as wootp:

            LOOKAHEAD = 3

            def attend(h, xh, ship=None):
                qn_h = qn0 if h == 0 else qn1
                qr_h = qt1r if h == 0 else qr1

                def finish(sb, psx, psdt):
                    # softmax normalize for a finished chunk; emitted a few
                    # matmuls into the NEXT chunk so the in-order PE never
                    # stalls on the (slow, single-lane) reciprocal
                    sl = slice(sb * 512, (sb + 1) * 512)
                    rd = asmp.tile([1, 512], f32, tag="rd")
                    nc.vector.reciprocal(rd[:], psdt[0:1, :])
                    psb2 = apsd.tile([128, 512], f32, tag="dn")
                    mm(psb2, ones_row, rd, True, True)
                    rdb = asmp.tile([128, 512], f32, tag="rdb")
                    nc.vector.tensor_copy(rdb[:], psb2[:])
                    nc.vector.tensor_tensor(xh[:, sl], psx[:], rdb[:],
                                            OP.mult)
                    if ship is not None:
                        ship(sb)

                pending = None
                for sb in range(4):
                    sl = slice(sb * 512, (sb + 1) * 512)
                    psx = apsx.tile([128, 512], f32, tag="apsx")
                    psdt = apsd.tile([128, 512], f32, tag="dn")
                    psd = psdt[0:1, :]
                    exq = []

                    def consume(tb, ex):
                        mm(psx, vt[tb][:, h * 128:(h + 1) * 128], ex,
                           tb == 0, tb == 15)
                        mm(psd, ones_col, ex, tb == 0, tb == 15)

                    for tb in range(16):
                        tsl = slice(tb * 128, (tb + 1) * 128)
                        pss = apss.tile([128, 512], f32, tag="apss")
                        mm(pss, kn[h][:, tsl], qn_h[:, sl], True, False)
                        mm(pss, kpe_sb[:, tsl], qr_h[:, sl], False, True)
                        if has_mask:
                            mk = amkp.tile([128, 512], b16, tag="amk")
                            nc.sync.dma_start(
                                mk[:], maskT.ap()[tsl, sl])
                            nc.vector.tensor_tensor(pss[:], pss[:], mk[:],
                                                    OP.add)
                        ex = aexp.tile([128, 512], b16, tag="aex")
                        nc.scalar.activation(ex[:], pss[:], AF.Exp)
                        exq.append((tb, ex))
                        if tb == 4 and pending is not None:
                            finish(*pending)
                            pending = None
                        if len(exq) > LOOKAHEAD:
                            consume(*exq.pop(0))
                    for item in exq:
                        consume(*item)
                    pending = (sb, psx, psdt)
                finish(*pending)

            def ship_h0(sb):
                if sb in (2, 3):
                    i = sb - 2
                    hs = slice(0, 1536) if i == 0 else slice(1536, S)
                    nc.sync.dma_start(bounce2a[i].ap(), xh0[:, hs])
                    nc.gpsimd.collective_compute(
                        "AllGather", OP.bypass, replica_groups=RG,
                        ins=[bounce2a[i].ap().opt()],
                        outs=[gath2a[i].ap().opt()])

            attend(0, xh0, ship=ship_h0)

            # head-1 attention runs while gath2a lands + wo half-accumulates
            def ship_h1(sb):
                if sb in (2, 3):
                    i = sb - 2
                    hs = slice(0, B2SPLIT) if i == 0 else slice(B2SPLIT, S)
                    nc.sync.dma_start(bounce2b[i].ap(), xh1[:, hs])
                    nc.gpsimd.collective_compute(
                        "AllGather", OP.bypass, replica_groups=RG,
                        ins=[bounce2b[i].ap().opt()],
                        outs=[gath2b[i].ap().opt()])

            attend(1, xh1, ship=ship_h1)

            xe_a = wop.tile([128, 8, S], b16)
            opart = [wop.tile([128, SSH], f32, name=f"op{st}")
                     for st in range(16)]
            for i in range(2):
                hs = slice(0, 1536) if i == 0 else slice(1536, S)
                for r in range(NCORES):
                    nc.sync.dma_start(xe_a[:, r, hs], gath2a[i].ap()[r])
                for st in (range(12) if i == 0 else range(12, 16)):
                    pso = wops.tile([128, SSH], f32, tag="wops")
                    for r in range(NCORES):
                        mm(pso, xe_a[:, r, st * 128:(st + 1) * 128],
                           wot_sb[:, 2 * r, :], r == 0, r == 7)
                    nc.vector.tensor_copy(opart[st][:], pso[:])

            xe_b = wop.tile([128, 8, S], b16)
            for i in range(2):
                hs = slice(0, B2SPLIT) if i == 0 else slice(B2SPLIT, S)
                for r in range(NCORES):
                    nc.sync.dma_start(xe_b[:, r, hs], gath2b[i].ap()[r])
                for st in (range(12) if i == 0 else range(12, 16)):
                    pso = wops.tile([128, SSH], f32, tag="wops")
                    for r in range(NCORES):
                        mm(pso, xe_b[:, r, st * 128:(st + 1) * 128],
                           wot_sb[:, 2 * r + 1, :], r == 0, r == 7)
                    ot = wootp.tile([128, SSH], f32, tag="ot")
                    nc.vector.tensor_tensor(ot[:], pso[:], opart[st][:],
                                            OP.add)
                    nc.sync.dma_start(out.ap()[st * 128:(st + 1) * 128, :],
                                      ot[:])

    nc.compile()
    return nc


def _prep_inputs(hidden_states, cos, sin, attn_mask, wq_a, q_norm_w, wq_b,
                 wkv_a, kv_norm_w, wkv_b, wo, has_mask):
    import ml_dtypes
    bf = ml_dtypes.bfloat16

    def c(x):
        return np.ascontiguousarray(x.astype(bf))

    hid = np.asarray(hidden_states, np.float32)[0]          # [S, H]
    hidT = c(hid.T.reshape(16, 128, S))                     # [H, S]
    A_T = np.vstack([np.asarray(wq_a, np.float32),
                     np.asarray(wkv_a, np.float32)]).T      # [H, CTOT]
    a_pe = c(A_T[:, 2048:2112].reshape(16, 128, 64))

    cosT = np.asarray(cos, np.float32).T                    # [64, S]
    sinT = np.asarray(sin, np.float32).T
    sinTs = sinT.copy()
    sinTs[0:32] *= -1.0
    cosT2 = c(np.concatenate([cosT, cosT], 0))              # [128, S]
    sinT2s = c(np.concatenate([sinTs, sinTs], 0))

    wqb = np.asarray(wq_b, np.float32) * np.asarray(q_norm_w, np.float32)[None]
    wqb = wqb * SOFTMAX_SCALE
    wkvb = (np.asarray(wkv_b, np.float32)
            * np.asarray(kv_norm_w, np.float32)[None])
    woT_full = np.asarray(wo, np.float32).T                 # [NH*DV, H]

    qperm = np.r_[0:128, 128:192, 320:384, 192:320]
    kvperm = np.r_[0:128, 256:384, 128:256, 384:512]

    # partial-sumsq row p: slab i=p//8, core r=p%8 -> global channel tile
    # 2r+i (tiles 0..11 are hq, 12..15 kv); fold the 1/D mean into the
    # selector values
    selH = np.zeros((16, 1), np.float32)
    selC = np.zeros((16, 1), np.float32)
    for p in range(16):
        tile_id = 2 * (p % 8) + p // 8
        if tile_id < 12:
            selH[p, 0] = 1.0 / Q_LORA
        else:
            selC[p, 0] = 1.0 / KV_LORA

    in_maps = []
    for r in range(NCORES):
        m = {
            "hidT": hidT,
            "a_own": c(A_T[:, r * 256:(r + 1) * 256].reshape(16, 128, 256)),
            "a_pe": a_pe,
            "cosT2": cosT2,
            "sinT2s": sinT2s,
            "wqbT": c(wqb[r * 384:(r + 1) * 384].T[:, qperm]
                      .reshape(12, 128, 384)),
            "wkvbT": c(wkvb[r * 512:(r + 1) * 512].T[:, kvperm]
                       .reshape(4, 128, 512)),
            "woT": c(woT_full[:, r * SSH:(r + 1) * SSH].reshape(16, 128, SSH)),
            "ones_a": np.ones((128, 1), bf),
            "ones_bf": np.ones((1, 128), np.float32),
            "selH": c(selH),
            "eye": np.eye(128, dtype=bf),
            "selC": c(selC),
        }
        if has_mask:
            m["maskT"] = c(np.asarray(attn_mask, np.float32).T)
        in_maps.append(m)
    return in_maps


def kernel(**inputs):
    from concourse.bass_utils import run_bass_kernel_spmd

    has_mask = bool(np.any(np.asarray(inputs["attn_mask"])))
    if has_mask not in _CACHE:
        _CACHE[has_mask] = _build(has_mask)
    nc = _CACHE[has_mask]

    in_maps = _prep_inputs(has_mask=has_mask, **inputs)
    res = run_bass_kernel_spmd(nc, in_maps, list(range(NCORES))).results
    full = np.concatenate([res[r]["out"] for r in range(NCORES)], axis=1)
    return full.reshape(B, S, H).astype(np.float32)


# revision 33
# speedup vs baseline: 1.0247x; 1.0026x over previous
"""MLA (DeepSeek-style multi-head latent attention) Bass kernel for 8 trn2 NeuronCores.

Sharding: tensor-parallel over heads (2 heads/core) for the big projections +
attention. The low-rank A-projections are CHANNEL-sharded (each core computes
256 of the 2048 hq+kv latent channels for all 2048 tokens) so the AllGathered
latents read back as contiguous 4KB runs; k_pe (64 rope channels) is computed
redundantly on every core, skipping it in the collective. RMS normalization
happens after the gather: per-core partial sum-of-squares rides the AllGather
as one extra bf16 row per channel block, and the rsqrt factors are folded into
the projection PSUM->SBUF copies (free-dim broadcast for q/k, per-partition
tensor_scalar for v). The output projection is column-parallel with a split
AllGather: head 0's attention output gathers and partially accumulates into
wo while head 1's attention still runs.

All tensors bf16 (fp32 PSUM accumulation); exp softmax without max-subtract.
Host-side (free) preprocessing: bf16 casts, weight transposes/permutations,
folding q_norm_w/kv_norm_w and SOFTMAX_SCALE into wq_b/wkv_b, rope sign folds.
"""

import math
import sys

import numpy as np

for _p in ("/opt/trn_rl_repo", "/root/.axon_site/_ro/trn_rl_repo"):
    if _p not in sys.path:
        sys.path.append(_p)

B, S, H = 1, 2048, 2048
NH = 16
Q_LORA, KV_LORA = 1536, 512
D_NOPE, D_ROPE, D_V = 128, 64, 128
D_QK = D_NOPE + D_ROPE
ROPE_FACTOR, MSCALE = 4.0, 1.0
SOFTMAX_SCALE = D_QK ** -0.5 * (0.1 * MSCALE * math.log(ROPE_FACTOR) + 1.0) ** 2
EPS = 1e-6

NCORES = 8
SSH = S // NCORES          # 256 output channels per core (wo column-parallel)
CTOT = Q_LORA + KV_LORA + D_ROPE   # 2112 latent channels

_CACHE = {}


def _build(has_mask: bool):
    import concourse.bacc as bacc
    import concourse.mybir as mybir
    import concourse.tile as tile

    f32 = mybir.dt.float32
    b16 = mybir.dt.bfloat16
    AF = mybir.ActivationFunctionType
    OP = mybir.AluOpType

    nc = bacc.Bacc("TRN2", target_bir_lowering=False, debug=False,
                   num_devices=NCORES)

    hidT = nc.dram_tensor("hidT", [16, 128, S], b16, kind="ExternalInput")
    a_own = nc.dram_tensor("a_own", [16, 128, 256], b16, kind="ExternalInput")
    a_pe = nc.dram_tensor("a_pe", [16, 128, 64], b16, kind="ExternalInput")
    cosT2 = nc.dram_tensor("cosT2", [128, S], b16, kind="ExternalInput")
    sinT2s = nc.dram_tensor("sinT2s", [128, S], b16, kind="ExternalInput")
    wqbT = nc.dram_tensor("wqbT", [12, 128, 384], b16, kind="ExternalInput")
    wkvbT = nc.dram_tensor("wkvbT", [4, 128, 512], b16, kind="ExternalInput")
    woT = nc.dram_tensor("woT", [16, 128, SSH], b16, kind="ExternalInput")
    ones_a = nc.dram_tensor("ones_a", [128, 1], b16, kind="ExternalInput")
    ones_bf = nc.dram_tensor("ones_bf", [1, 128], f32, kind="ExternalInput")
    selH_d = nc.dram_tensor("selH", [16, 1], b16, kind="ExternalInput")
    eye_d = nc.dram_tensor("eye", [128, 128], b16, kind="ExternalInput")
    selC_d = nc.dram_tensor("selC", [16, 1], b16, kind="ExternalInput")
    if has_mask:
        maskT = nc.dram_tensor("maskT", [S, S], b16, kind="ExternalInput")
    out = nc.dram_tensor("out", [SSH, S], f32, kind="ExternalOutput")

    bounce1 = [nc.dram_tensor(f"bounce1{i}", [129, S], b16)
               for i in range(2)]
    gath1 = [nc.dram_tensor(f"gath1{i}", [NCORES, 129, S], b16,
                            addr_space="Shared") for i in range(2)]
    bounce2a = [nc.dram_tensor("bounce2a0", [128, 1536], b16),
                nc.dram_tensor("bounce2a1", [128, S - 1536], b16)]
    gath2a = [nc.dram_tensor("gath2a0", [NCORES, 128, 1536], b16,
                             addr_space="Shared"),
              nc.dram_tensor("gath2a1", [NCORES, 128, S - 1536], b16,
                             addr_space="Shared")]
    B2SPLIT = 1536
    bounce2b = [nc.dram_tensor("bounce2b0", [128, B2SPLIT], b16),
                nc.dram_tensor("bounce2b1", [128, S - B2SPLIT], b16)]
    gath2b = [nc.dram_tensor("gath2b0", [NCORES, 128, B2SPLIT], b16,
                             addr_space="Shared"),
              nc.dram_tensor("gath2b1", [NCORES, 128, S - B2SPLIT], b16,
                             addr_space="Shared")]

    RG = [list(range(NCORES))]

    def mm(ps, lhsT, rhs, start, stop):
        nc.tensor.matmul(ps, lhsT, rhs, start=start, stop=stop)

    from contextlib import ExitStack
    with tile.TileContext(nc) as tc, ExitStack() as _st:
        constp = _st.enter_context(tc.tile_pool(name="const", bufs=1))
        ones_col = constp.tile([128, 1], b16)
        nc.sync.dma_start(ones_col[:], ones_a.ap())
        ones_row = constp.tile([1, 128], f32)
        nc.sync.dma_start(ones_row[:], ones_bf.ap())
        selh_sb = constp.tile([16, 1], b16)
        nc.sync.dma_start(selh_sb[:], selH_d.ap())
        selc_sb = constp.tile([16, 1], b16)
        nc.sync.dma_start(selc_sb[:], selC_d.ap())
        eye_sb = constp.tile([128, 128], b16)
        nc.sync.dma_start(eye_sb[:], eye_d.ap())
        eps1 = constp.tile([1, 1], f32)
        nc.any.memset(eps1[:], EPS)
        eps_col = constp.tile([128, 1], f32)
        nc.any.memset(eps_col[:], EPS)

        # stage-1 weight tiles (DMAs issued after stage-0's loads)
        s1wp = _st.enter_context(tc.tile_pool(name="s1w", bufs=1))
        wqb_sb = s1wp.tile([128, 12, 384], b16)
        wkvb_sb = s1wp.tile([128, 4, 512], b16)
        cos2_sb = s1wp.tile([128, S], b16)
        sin2_sb = s1wp.tile([128, S], b16)
        wot_sb = s1wp.tile([128, 16, SSH], b16)   # DMA issued before attention

        # attention-lifetime pool (also covers the wo epilogue)
        attp = _st.enter_context(tc.tile_pool(name="att", bufs=1))
        kpe_sb = attp.tile([128, S], b16)
        nc.any.memset(kpe_sb[64:128, :], 0.0)

        # ---------------- stage 0: own 256 latent channels for all tokens
        with tc.tile_pool(name="s0", bufs=1) as s0p, \
             tc.tile_pool(name="s0ps", bufs=1, space="PSUM") as s0ps, \
             tc.tile_pool(name="s0ss", bufs=2, space="PSUM") as s0ssp, \
             tc.tile_pool(name="s0pe", bufs=2, space="PSUM") as s0pe, \
             tc.tile_pool(name="s0sq", bufs=3) as s0sqp:
            a_sb = s0p.tile([128, 16, 256], b16)
            hid_sb = s0p.tile([128, 16, S], b16)
            for g in range(8):
                gs = slice(g * 2, (g + 1) * 2)
                nc.sync.dma_start(
                    a_sb[:, gs, :],
                    a_own.ap()[gs].rearrange("o p c -> p o c"))
                nc.sync.dma_start(
                    hid_sb[:, gs, :],
                    hidT.ap()[gs].rearrange("o p s -> p o s"))
            ape_sb = s0p.tile([128, 16, 64], b16)
            nc.sync.dma_start(ape_sb[:], a_pe.ap().rearrange("o p c -> p o c"))
            # stage-1 weights load behind stage-0's operands
            nc.sync.dma_start(cos2_sb[:], cosT2.ap())
            nc.sync.dma_start(sin2_sb[:], sinT2s.ap())
            nc.sync.dma_start(wqb_sb[:],
                              wqbT.ap().rearrange("o p d -> p o d"))
            nc.sync.dma_start(wkvb_sb[:],
                              wkvbT.ap().rearrange("o p d -> p o d"))

            raw = s0p.tile([128, 2, S], b16)
            kpraw = s0p.tile([64, S], b16)
            ssb16 = [s0p.tile([1, S], b16, name=f"ssb{i}")
                     for i in range(2)]
            for ct in range(2):
                # hb-major accumulation into 4 open PSUM groups: matmuls
                # consume each arriving hid chunk immediately
                pss0 = [s0ps.tile([128, 512], f32, name=f"s0ps{tt}")
                        for tt in range(4)]
                for hb in range(16):
                    for tt in range(4):
                        mm(pss0[tt], a_sb[:, hb, ct * 128:(ct + 1) * 128],
                           hid_sb[:, hb, tt * 512:(tt + 1) * 512],
                           hb == 0, hb == 15)
                for tt in range(4):
                    sl = slice(tt * 512, (tt + 1) * 512)
                    ps = pss0[tt]
                    nc.vector.tensor_copy(raw[:, ct, sl], ps[:])
                    sq = s0sqp.tile([128, 512], b16, tag="s0sq")
                    nc.scalar.activation(sq[:], ps[:], AF.Square)
                    ssp = s0ssp.tile([1, 512], f32, tag="ss")
                    mm(ssp, ones_col, sq, True, True)
                    nc.scalar.copy(ssb16[ct][:, sl], ssp[:])
                    nc.sync.dma_start(
                        bounce1[ct].ap()[0:128, sl], raw[:, ct, sl])
                # ship this channel slab; the second overlaps consumption
                nc.sync.dma_start(bounce1[ct].ap()[128:129, :],
                                  ssb16[ct][:])
                nc.gpsimd.collective_compute(
                    "AllGather", OP.bypass, replica_groups=RG,
                    ins=[bounce1[ct].ap().opt()],
                    outs=[gath1[ct].ap().opt()])

            # k_pe matmuls after the ship: they fill the AllGather wait
            for tt in range(4):
                sl = slice(tt * 512, (tt + 1) * 512)
                kp = s0pe.tile([64, 512], f32, tag="kpe")
                for hb in range(16):
                    mm(kp, ape_sb[:, hb, :], hid_sb[:, hb, sl],
                       hb == 0, hb == 15)
                nc.vector.tensor_copy(kpraw[:, sl], kp[:])

            # k_pe rope (redundant on every core; not in the collective)
            t1 = s0p.tile([64, S], b16)
            nc.vector.tensor_tensor(t1[:], kpraw[:], cos2_sb[0:64, :], OP.mult)
            rsw = s0p.tile([64, S], b16)
            nc.sync.dma_start(rsw[0:32], kpraw[32:64])
            nc.sync.dma_start(rsw[32:64], kpraw[0:32])
            nc.vector.tensor_tensor(rsw[:], rsw[:], sin2_sb[0:64, :], OP.mult)
            nc.vector.tensor_tensor(kpe_sb[0:64, :], t1[:], rsw[:], OP.add)

        # ---------------- stage 1: per-head projections + attention
        qn0 = attp.tile([128, S], b16)
        qt1 = attp.tile([128, S], b16)
        qn1 = attp.tile([128, S], b16)
        qdst = (qn0, qt1, qn1)
        kn0 = attp.tile([128, S], b16)
        kn1 = attp.tile([128, S], b16)
        kn = (kn0, kn1)
        vt = [attp.tile([128, 256], b16, name=f"vt{tb}") for tb in range(16)]
        qt1r = attp.tile([128, S], b16)
        qr1 = attp.tile([128, S], b16)
        nc.any.memset(qr1[64:128, :], 0.0)
        xh0 = attp.tile([128, S], b16)
        xh1 = attp.tile([128, S], b16)

        with tc.tile_pool(name="proj", bufs=1) as prj, \
             tc.tile_pool(name="bcp", bufs=2) as bcp:
            # gathered reads: all contiguous 4KB runs
            # channel tile id: slab i row-block r -> global tile 2r+i.
            # hq_all/kv_all cc index == global tile id (hq 0..11, kv 0..3).
            partials = prj.tile([16, S], b16)
            hq_all = prj.tile([128, 12, S], b16)
            kv_all = prj.tile([128, 4, S], b16)
            for i in range(2):
                nc.sync.dma_start(
                    partials[8 * i:8 * i + 8, :],
                    gath1[i].ap()[:, 128, :])
                for r in (6, 7):
                    nc.sync.dma_start(
                        kv_all[:, 2 * (r - 6) + i, :],
                        gath1[i].ap()[r, 0:128, :])
                for tt in range(4):
                    sl = slice(tt * 512, (tt + 1) * 512)
                    for r in range(6):
                        nc.sync.dma_start(
                            hq_all[:, 2 * r + i, sl],
                            gath1[i].ap()[r, 0:128, sl])

            # projections with normalization fused into the PSUM->SBUF step
            with tc.tile_pool(name="p1ps", bufs=3, space="PSUM") as p1ps, \
                 tc.tile_pool(name="p1psv", bufs=1, space="PSUM") as p1psv:
                QEVEN = [0, 2, 4, 6, 8, 10]
                QODD = [1, 3, 5, 7, 9, 11]
                KCC = [0, 2, 1, 3]
                # pass A: even channel tiles (slab 0) accumulate to bf16
                # partials while slab 1 is still gathering
                qparts = {}
                for tt in range(4):
                    sl = slice(tt * 512, (tt + 1) * 512)
                    for m in range(3):
                        ps = p1ps.tile([128, 512], f32, tag="p1ps")
                        for j, cc in enumerate(QEVEN):
                            mm(ps, wqb_sb[:, cc, m * 128:(m + 1) * 128],
                               hq_all[:, cc, sl], j == 0, j == 5)
                        qp = prj.tile([128, 512], b16, name=f"qp{tt}_{m}")
                        nc.vector.tensor_copy(qp[:], ps[:])
                        qparts[(tt, m)] = qp

            # rms factors: [1, S] free-layout (q/k) + [128, 16] col-layout (v)
                rc_hq_t = prj.tile([1, S], f32)
                rc_kv_t = prj.tile([1, S], f32)
                sqcol = prj.tile([128, 16], f32)
                rckv_col = prj.tile([128, 16], f32)
                with tc.tile_pool(name="sqp", bufs=2) as sqp, \
                     tc.tile_pool(name="pfac", bufs=2, space="PSUM") as pfac:
                    for tt in range(4):
                        sl = slice(tt * 512, (tt + 1) * 512)
                        for selt, rct in ((selh_sb, rc_hq_t),
                                          (selc_sb, rc_kv_t)):
                            ps2 = pfac.tile([1, 512], f32, tag="ps2")
                            mm(ps2, selt, partials[:, sl], True, True)
                            sqt = sqp.tile([1, 512], f32, tag="sq")
                            nc.scalar.activation(sqt[:], ps2[:], AF.Sqrt,
                                                 bias=eps1[:])
                            nc.vector.reciprocal(rct[:, sl], sqt[:])
                    pscol = pfac.tile([128, 16], f32, tag="pscol")
                    for tb in range(16):
                        mm(pscol[:, tb:tb + 1],
                           partials[:, tb * 128:(tb + 1) * 128],
                           selc_sb, True, True)
                    nc.scalar.activation(sqcol[:], pscol[:], AF.Sqrt,
                                         bias=eps_col[:])
                    nc.vector.reciprocal(rckv_col[:], sqcol[:])

                # pass B: identity re-injection + odd channel tiles, with
                # the rsqrt broadcasts produced on demand per token chunk
                with tc.tile_pool(name="pbc", bufs=2, space="PSUM") as pbc:
                    for tt in range(4):
                        sl = slice(tt * 512, (tt + 1) * 512)
                        bts = []
                        for ty, rct in ((0, rc_hq_t), (1, rc_kv_t)):
                            psb = pbc.tile([128, 512], f32, tag="pbc")
                            mm(psb, ones_row, rct[:, sl], True, True)
                            bt = bcp.tile([128, 512], f32, tag=f"bc{ty}")
                            nc.vector.tensor_copy(bt[:], psb[:])
                            bts.append(bt)
                        bchq_t, bckv_t = bts
                        for m in range(3):
                            ps = p1ps.tile([128, 512], f32, tag="p1ps")
                            mm(ps, eye_sb, qparts[(tt, m)][:], True, False)
                            for j, cc in enumerate(QODD):
                                mm(ps, wqb_sb[:, cc, m * 128:(m + 1) * 128],
                                   hq_all[:, cc, sl], False, j == 5)
                            nc.vector.tensor_tensor(qdst[m][:, sl], ps[:],
                                                    bchq_t[:], OP.mult)
                        for kh in range(2):
                            ps = p1ps.tile([128, 512], f32, tag="p1ps")
                            for j, cc in enumerate(KCC):
                                mm(ps, wkvb_sb[:, cc,
                                               kh * 128:(kh + 1) * 128],
                                   kv_all[:, cc, sl], j == 0, j == 3)
                            nc.vector.tensor_tensor(kn[kh][:, sl], ps[:],
                                                    bckv_t[:], OP.mult)
                for tb in range(16):
                    ps = p1psv.tile([128, 256], f32, tag="p1psv")
                    for j, cc in enumerate(KCC):
                        mm(ps, kv_all[:, cc, tb * 128:(tb + 1) * 128],
                           wkvb_sb[:, cc, 256:512], j == 0, j == 3)
                        # lhsT = latent chunk [c,t], rhs = v cols of wkv_b'^T
                    nc.vector.tensor_scalar_mul(vt[tb][:], ps[:],
                                                rckv_col[:, tb:tb + 1])

                # rope on q (both heads share qt1: rows 0:64 h0, 64:128 h1)
                tmp = xh1  # scratch; attention writes xh1 later
                for b in (0, 64):
                    nc.sync.dma_start(tmp[b:b + 32], qt1[b + 32:b + 64])
                    nc.sync.dma_start(tmp[b + 32:b + 64], qt1[b:b + 32])
                nc.vector.tensor_tensor(qt1r[:], qt1[:], cos2_sb[:], OP.mult)
                nc.vector.tensor_tensor(tmp[:], tmp[:], sin2_sb[:], OP.mult)
                nc.vector.tensor_tensor(qt1r[:], qt1r[:], tmp[:], OP.add)
                # h1 rope rows to base-0 tile (rows 64: zero; kpe rows 64: 0)
                nc.sync.dma_start(qr1[0:64, :], qt1r[64:128])

        # wo weights prefetch (overlaps attention)
        nc.sync.dma_start(wot_sb[:], woT.ap().rearrange("o p s -> p o s"))

        # attention + split AllGather + column-parallel wo
        with tc.tile_pool(name="apss", bufs=3, space="PSUM") as apss, \
             tc.tile_pool(name="apsx", bufs=2, space="PSUM") as apsx, \
             tc.tile_pool(name="apsd", bufs=2, space="PSUM") as apsd, \
             tc.tile_pool(name="wops", bufs=1, space="PSUM") as wops, \
             tc.tile_pool(name="aex", bufs=24) as aexp, \
             tc.tile_pool(name="asm", bufs=2) as asmp, \
             tc.tile_pool(name="amk", bufs=2) as amkp, \
             tc.tile_pool(name="wop", bufs=1) as wop, \
             tc.tile_pool(name="woot", bufs=3) as wootp:

            def attend(h, xh, ship=None):
                qn_h = qn0 if h == 0 else qn1
                qr_h = qt1r if h == 0 else qr1

                def finish(sb, psx, psdt):
                    # softmax normalize for a finished chunk; emitted a few
                    # matmuls into the NEXT chunk so the in-order PE never
                    # stalls on the (slow, single-lane) reciprocal
                    sl = slice(sb * 512, (sb + 1) * 512)
                    rd = asmp.tile([1, 512], f32, tag="rd")
                    nc.vector.reciprocal(rd[:], psdt[0:1, :])
                    psb2 = apsd.tile([128, 512], f32, tag="dn")
                    mm(psb2, ones_row, rd, True, True)
                    rdb = asmp.tile([128, 512], f32, tag="rdb")
                    nc.vector.tensor_copy(rdb[:], psb2[:])
                    nc.vector.tensor_tensor(xh[:, sl], psx[:], rdb[:],
                                            OP.mult)
                    if ship is not None:
                        ship(sb)

                pending = None
                for sb in range(4):
                    sl = slice(sb * 512, (sb + 1) * 512)
                    psx = apsx.tile([128, 512], f32, tag="apsx")
                    psdt = apsd.tile([128, 512], f32, tag="dn")
                    psd = psdt[0:1, :]
                    exs = []
                    # phase 1: all score matmuls + exp
                    for tb in range(16):
                        tsl = slice(tb * 128, (tb + 1) * 128)
                        pss = apss.tile([128, 512], f32, tag="apss")
                        mm(pss, kn[h][:, tsl], qn_h[:, sl], True, False)
                        mm(pss, kpe_sb[:, tsl], qr_h[:, sl], False, True)
                        if has_mask:
                            mk = amkp.tile([128, 512], b16, tag="amk")
                            nc.sync.dma_start(
                                mk[:], maskT.ap()[tsl, sl])
                            nc.vector.tensor_tensor(pss[:], pss[:], mk[:],
                                                    OP.add)
                        ex = aexp.tile([128, 512], b16, tag="aex")
                        nc.scalar.activation(ex[:], pss[:], AF.Exp)
                        exs.append(ex)
                        if tb == 4 and pending is not None:
                            finish(*pending)
                            pending = None
                    # phase 2: attention @ v (one stationary switch per tb)
                    for tb in range(16):
                        mm(psx, vt[tb][:, h * 128:(h + 1) * 128], exs[tb],
                           tb == 0, tb == 15)
                    # phase 3: denominators (stationary ones-vector persists)
                    for tb in range(16):
                        mm(psd, ones_col, exs[tb], tb == 0, tb == 15)
                    pending = (sb, psx, psdt)
                finish(*pending)

            def ship_h0(sb):
                if sb in (2, 3):
                    i = sb - 2
                    hs = slice(0, 1536) if i == 0 else slice(1536, S)
                    nc.sync.dma_start(bounce2a[i].ap(), xh0[:, hs])
                    nc.gpsimd.collective_compute(
                        "AllGather", OP.bypass, replica_groups=RG,
                        ins=[bounce2a[i].ap().opt()],
                        outs=[gath2a[i].ap().opt()])

            attend(0, xh0, ship=ship_h0)

            # head-1 attention runs while gath2a lands + wo half-accumulates
            def ship_h1(sb):
                if sb in (2, 3):
                    i = sb - 2
                    hs = slice(0, B2SPLIT) if i == 0 else slice(B2SPLIT, S)
                    nc.sync.dma_start(bounce2b[i].ap(), xh1[:, hs])
                    nc.gpsimd.collective_compute(
                        "AllGather", OP.bypass, replica_groups=RG,
                        ins=[bounce2b[i].ap().opt()],
                        outs=[gath2b[i].ap().opt()])

            attend(1, xh1, ship=ship_h1)

            # transposed wo: out^T[h_col, tok] so matmuls stream 512 wide
            xe_a = wop.tile([128, 8, S], b16)
            opart = [wop.tile([128, 512], f32, name=f"op{g}")
                     for g in range(8)]
            for i, tts in ((0, (0, 1, 2)), (1, (3,))):
                hs = slice(0, 1536) if i == 0 else slice(1536, S)
                for r in range(NCORES):
                    nc.sync.dma_start(xe_a[:, r, hs], gath2a[i].ap()[r])
                for tt in tts:
                    tsl = slice(tt * 512, (tt + 1) * 512)
                    for ho in range(2):
                        pso = wops.tile([128, 512], f32, tag="wops")
                        for r in range(NCORES):
                            mm(pso, wot_sb[:, 2 * r,
                                           ho * 128:(ho + 1) * 128],
                               xe_a[:, r, tsl], r == 0, r == 7)
                        nc.vector.tensor_copy(opart[2 * tt + ho][:], pso[:])

            xe_b = wop.tile([128, 8, S], b16)
            for i, tts in ((0, (0, 1, 2)), (1, (3,))):
                hs = slice(0, B2SPLIT) if i == 0 else slice(B2SPLIT, S)
                for r in range(NCORES):
                    nc.sync.dma_start(xe_b[:, r, hs], gath2b[i].ap()[r])
                for tt in tts:
                    tsl = slice(tt * 512, (tt + 1) * 512)
                    for ho in range(2):
                        pso = wops.tile([128, 512], f32, tag="wops")
                        for r in range(NCORES):
                            mm(pso, wot_sb[:, 2 * r + 1,
                                           ho * 128:(ho + 1) * 128],
                               xe_b[:, r, tsl], r == 0, r == 7)
                        ot = wootp.tile([128, 512], f32, tag="ot")
                        nc.vector.tensor_tensor(ot[:], pso[:],
                                                opart[2 * tt + ho][:],
                                                OP.add)
                        nc.sync.dma_start(
                            out.ap()[ho * 128:(ho + 1) * 128, tsl], ot[:])

    nc.compile()
    return nc


def _prep_inputs(hidden_states, cos, sin, attn_mask, wq_a, q_norm_w, wq_b,
                 wkv_a, kv_norm_w, wkv_b, wo, has_mask):
    import ml_dtypes
    bf = ml_dtypes.bfloat16

    def c(x):
        return np.ascontiguousarray(x.astype(bf))

    hid = np.asarray(hidden_states, np.float32)[0]          # [S, H]
    hidT = c(hid.T.reshape(16, 128, S))                     # [H, S]
    A_T = np.vstack([np.asarray(wq_a, np.float32),
                     np.asarray(wkv_a, np.float32)]).T      # [H, CTOT]
    a_pe = c(A_T[:, 2048:2112].reshape(16, 128, 64))

    cosT = np.asarray(cos, np.float32).T                    # [64, S]
    sinT = np.asarray(sin, np.float32).T
    sinTs = sinT.copy()
    sinTs[0:32] *= -1.0
    cosT2 = c(np.concatenate([cosT, cosT], 0))              # [128, S]
    sinT2s = c(np.concatenate([sinTs, sinTs], 0))

    wqb = np.asarray(wq_b, np.float32) * np.asarray(q_norm_w, np.float32)[None]
    wqb = wqb * SOFTMAX_SCALE
    wkvb = (np.asarray(wkv_b, np.float32)
            * np.asarray(kv_norm_w, np.float32)[None])
    woT_full = np.asarray(wo, np.float32).T                 # [NH*DV, H]

    qperm = np.r_[0:128, 128:192, 320:384, 192:320]
    kvperm = np.r_[0:128, 256:384, 128:256, 384:512]

    # partial-sumsq row p: slab i=p//8, core r=p%8 -> global channel tile
    # 2r+i (tiles 0..11 are hq, 12..15 kv); fold the 1/D mean into the
    # selector values
    selH = np.zeros((16, 1), np.float32)
    selC = np.zeros((16, 1), np.float32)
    for p in range(16):
        tile_id = 2 * (p % 8) + p // 8
        if tile_id < 12:
            selH[p, 0] = 1.0 / Q_LORA
        else:
            selC[p, 0] = 1.0 / KV_LORA

    in_maps = []
    for r in range(NCORES):
        m = {
            "hidT": hidT,
            "a_own": c(A_T[:, r * 256:(r + 1) * 256].reshape(16, 128, 256)),
            "a_pe": a_pe,
            "cosT2": cosT2,
            "sinT2s": sinT2s,
            "wqbT": c(wqb[r * 384:(r + 1) * 384].T[:, qperm]
                      .reshape(12, 128, 384)),
            "wkvbT": c(wkvb[r * 512:(r + 1) * 512].T[:, kvperm]
                       .reshape(4, 128, 512)),
            "woT": c(woT_full[:, r * SSH:(r + 1) * SSH].reshape(16, 128, SSH)),
            "ones_a": np.ones((128, 1), bf),
            "ones_bf": np.ones((1, 128), np.float32),
            "selH": c(selH),
            "eye": np.eye(128, dtype=bf),
            "selC": c(selC),
        }
        if has_mask:
            m["maskT"] = c(np.asarray(attn_mask, np.float32).T)
        in_maps.append(m)
    return in_maps


def kernel(**inputs):
    from concourse.bass_utils import run_bass_kernel_spmd

    has_mask = bool(np.any(np.asarray(inputs["attn_mask"])))
    if has_mask not in _CACHE:
        _CACHE[has_mask] = _build(has_mask)
    nc = _CACHE[has_mask]

    in_maps = _prep_inputs(has_mask=has_mask, **inputs)
    res = run_bass_kernel_spmd(nc, in_maps, list(range(NCORES))).results
    full = np.concatenate([res[r]["out"].T for r in range(NCORES)], axis=1)
    return full.reshape(B, S, H).astype(np.float32)
